# revision 10
# baseline (speedup 1.0000x reference)
"""MoE (8 experts, top-2) Trainium2 kernel — expert-parallel across 8 NeuronCores.

kernel(**inputs) takes FULL numpy inputs (as in setup_inputs()) and returns
(output[2,2048,1024] f32, aux_loss f32), matching the reference. Core r holds
expert r's weights; x and the router are replicated. Each core computes
gate_r(t) * FFN_r(x_t) for all tokens; a ReduceScatter sums the partials so
core r ends with output rows [512r : 512(r+1)). The host concatenates.

Numerics: FFN matmuls in bf16 (f32 accumulate). Router in f32 — bf16 routing
flips ~25 tokens' top-2 and blows the error budget (measured 4.8e-2 vs 3.2e-3).
"""

import numpy as np

import concourse.bass as bass
import concourse.bacc as bacc
import concourse.mybir as mybir
import concourse.tile as tile
from concourse.bass_utils import run_bass_kernel_spmd
from concourse.masks import make_identity

B, S, D, F, E, TOPK = 2, 2048, 1024, 4096, 8, 2
T = B * S               # 4096 tokens
NCORES = 8
CHUNK = 1024            # tokens per pipeline chunk
NCHUNK = T // CHUNK     # 4
TPT = CHUNK // 128      # 8 token tiles per chunk
KD = D // 128           # 8 k-tiles over d_model
MF = F // 128           # 32 m-tiles over d_ff
ND2 = D // 512          # 2 n-chunks of mm2 output
TG = 4                  # token tiles per mm2 psum group (4 psum banks)

FP32 = mybir.dt.float32
BF16 = mybir.dt.bfloat16
AF = mybir.ActivationFunctionType
ALU = mybir.AluOpType


def build_moe_kernel():
    nc = bacc.Bacc("TRN2")

    x_d = nc.declare_dram_parameter("x", [T, D], FP32, isOutput=False)
    w1_d = nc.declare_dram_parameter("W1", [D, F], FP32, isOutput=False)
    b1_d = nc.declare_dram_parameter("b1", [F], FP32, isOutput=False)
    w2_d = nc.declare_dram_parameter("W2", [F, D], FP32, isOutput=False)
    b2f_d = nc.declare_dram_parameter("b2f", [128, D], FP32, isOutput=False)
    wr_d = nc.declare_dram_parameter("Wr", [D, E], FP32, isOutput=False)
    brf_d = nc.declare_dram_parameter("brf", [128, E], FP32, isOutput=False)
    esel_d = nc.declare_dram_parameter("esel", [128, E], FP32, isOutput=False)

    out_d = nc.declare_dram_parameter("out", [T // NCORES, D], FP32, isOutput=True)
    aux_d = nc.declare_dram_parameter("aux", [1, 1], FP32, isOutput=True)

    partial_d = nc.dram_tensor("partial", [T, D], FP32)
    rs_out_d = nc.dram_tensor("rs_out", [T // NCORES, D], FP32)
    # bf16 weight staging, laid out for contiguous per-tile streaming
    w1bf_d = nc.dram_tensor("w1bf", [MF, 128, KD, 128], BF16)   # [m][dm_p, kd, ff]
    w2bf_d = nc.dram_tensor("w2bf", [ND2, MF, 128, 512], BF16)  # [n2][m][ff_p, d]

    with tile.TileContext(nc) as tc:
        with (
            tc.tile_pool(name="const", bufs=1) as constp,
            tc.tile_pool(name="stage", bufs=2) as stage,
            tc.tile_pool(name="wstream", bufs=3) as wstream,
            tc.tile_pool(name="xpipe", bufs=2) as xpipe,
            tc.tile_pool(name="x32pipe", bufs=1) as x32pipe,
            tc.tile_pool(name="hbuf", bufs=1) as hbuf,
            tc.tile_pool(name="gatep", bufs=2) as gatep,
            tc.tile_pool(name="outp", bufs=3) as outp,
            tc.tile_pool(name="auxp", bufs=1) as auxp,
            tc.tile_pool(name="ps_t", bufs=1, space="PSUM") as ps_t,
            tc.tile_pool(name="ps_r", bufs=1, space="PSUM") as ps_r,
            tc.tile_pool(name="ps_1", bufs=2, space="PSUM") as ps_1,
            tc.tile_pool(name="ps_2", bufs=4, space="PSUM") as ps_2,
        ):
            # ---------- constants ----------
            ident = constp.tile([128, 128], FP32)
            make_identity(nc, ident)
            ones_b = constp.tile([1, 512], BF16)
            nc.vector.memset(ones_b, 1.0)
            ones32_col = constp.tile([128, 1], FP32)
            nc.vector.memset(ones32_col, 1.0)
            esel_sb = constp.tile([128, E], FP32)
            nc.sync.dma_start(esel_sb, esel_d[:])

            wr32 = constp.tile([128, KD, E], FP32)
            nc.sync.dma_start(wr32, wr_d.rearrange("(k p) e -> p k e", p=128))
            brf_sb = constp.tile([128, E], FP32)
            nc.sync.dma_start(brf_sb, brf_d[:])
            b1_part = constp.tile([128, MF], FP32)
            nc.sync.dma_start(b1_part, b1_d.rearrange("(m p) -> p m", p=128))
            b2f_sb = constp.tile([128, D], FP32)
            nc.sync.dma_start(b2f_sb, b2f_d[:])

            # ---------- cast weights to bf16 DRAM staging ----------
            w1r = w1_d.rearrange("(k p) f -> p k f", p=128)
            for k in range(KD):
                for h in range(2):
                    st = stage.tile([128, F // 2], FP32, tag="wstage")
                    nc.sync.dma_start(st, w1r[:, k, h * 2048:(h + 1) * 2048])
                    cb = stage.tile([128, F // 2], BF16, tag="wcast")
                    nc.vector.tensor_copy(cb, st)
                    # the 16 m-tiles covered by this half
                    m0 = h * 16
                    nc.sync.dma_start(
                        w1bf_d[m0:m0 + 16, :, k, :].rearrange("m p f -> p m f"),
                        cb.rearrange("p (m f) -> p m f", m=16),
                    )
            w2r = w2_d.rearrange("(m p) d -> p m d", p=128)
            for mp in range(0, MF, 2):
                st = stage.tile([128, 2, D], FP32, tag="wstage")
                nc.sync.dma_start(st, w2r[:, mp:mp + 2, :])
                cb = stage.tile([128, 2, D], BF16, tag="wcast")
                nc.scalar.activation(cb, st, AF.Copy)
                for n2 in range(ND2):
                    nc.sync.dma_start(
                        w2bf_d[n2, mp:mp + 2, :, :].rearrange("m p d -> p m d"),
                        cb[:, :, n2 * 512:(n2 + 1) * 512],
                    )

            imp_acc = auxp.tile([128, E], FP32)
            nc.vector.memset(imp_acc, 0.0)
            load_acc = auxp.tile([128, E], FP32)
            nc.vector.memset(load_acc, 0.0)

            for c in range(NCHUNK):
                # ---------- load + transpose chunk ----------
                xT = xpipe.tile([128, KD, CHUNK], BF16, tag="xT")
                xT32 = x32pipe.tile([128, KD, CHUNK], FP32, tag="xT32")
                for t in range(TPT):
                    row0 = c * CHUNK + t * 128
                    xst = stage.tile([128, D], FP32, tag="xstage")
                    nc.sync.dma_start(xst, x_d[row0:row0 + 128, :])
                    for k in range(KD):
                        pst = ps_t.tile([128, 128], FP32, tag="pst")
                        nc.tensor.transpose(pst, xst[:, k * 128:(k + 1) * 128], ident)
                        nc.vector.tensor_copy(xT32[:, k, t * 128:(t + 1) * 128], pst)
                        nc.scalar.activation(xT[:, k, t * 128:(t + 1) * 128], pst, AF.Copy)

                # ---------- router (f32) ----------
                logits = gatep.tile([128, TPT, E], FP32, tag="logits")
                for t in range(TPT):
                    psr = ps_r.tile([128, E], FP32, tag="psr")
                    for k in range(KD):
                        nc.tensor.matmul(
                            psr, xT32[:, k, t * 128:(t + 1) * 128], wr32[:, k, :],
                            start=(k == 0), stop=(k == KD - 1),
                        )
                    nc.vector.tensor_add(logits[:, t, :], psr, brf_sb)

                # ---------- top-2 + gate ----------
                m1 = gatep.tile([128, TPT, 1], FP32, tag="m1")
                nc.vector.tensor_reduce(m1, logits, mybir.AxisListType.X, ALU.max)
                mask1 = gatep.tile([128, TPT, E], FP32, tag="mask1")
                nc.vector.tensor_tensor(mask1, logits, m1.to_broadcast([128, TPT, E]), ALU.is_equal)
                lm = gatep.tile([128, TPT, E], FP32, tag="lm")
                nc.vector.tensor_scalar(lm, mask1, -1e30, 0.0, ALU.mult, ALU.add)
                nc.vector.tensor_add(lm, lm, logits)
                m2 = gatep.tile([128, TPT, 1], FP32, tag="m2")
                nc.vector.tensor_reduce(m2, lm, mybir.AxisListType.X, ALU.max)
                mask2 = gatep.tile([128, TPT, E], FP32, tag="mask2")
                nc.vector.tensor_tensor(mask2, logits, m2.to_broadcast([128, TPT, E]), ALU.is_equal)
                dlt = gatep.tile([128, TPT, 1], FP32, tag="dlt")
                nc.vector.tensor_tensor(dlt, m1, m2, ALU.subtract)
                p1 = gatep.tile([128, TPT, 1], FP32, tag="p1")
                nc.scalar.activation(p1, dlt, AF.Sigmoid)
                p2 = gatep.tile([128, TPT, 1], FP32, tag="p2")
                nc.vector.tensor_scalar(p2, p1, -1.0, 1.0, ALU.mult, ALU.add)
                gate = gatep.tile([128, TPT, E], FP32, tag="gate")
                gtmp = gatep.tile([128, TPT, E], FP32, tag="gtmp")
                nc.vector.tensor_tensor(gate, mask1, p1.to_broadcast([128, TPT, E]), ALU.mult)
                nc.vector.tensor_tensor(gtmp, mask2, p2.to_broadcast([128, TPT, E]), ALU.mult)
                nc.vector.tensor_add(gate, gate, gtmp)
                ge = gatep.tile([128, TPT], FP32, tag="ge")
                gsel = gatep.tile([128, TPT, E], FP32, tag="gsel")
                nc.vector.tensor_tensor(
                    gsel, gate, esel_sb[:, None, :].to_broadcast([128, TPT, E]), ALU.mult
                )
                nc.vector.tensor_reduce(ge, gsel, mybir.AxisListType.X, ALU.add)

                # ---------- aux-loss accumulation ----------
                negm1 = gatep.tile([128, TPT, 1], FP32, tag="negm1")
                nc.vector.tensor_scalar_mul(negm1, m1, -1.0)
                eexp = gatep.tile([128, TPT, E], FP32, tag="eexp")
                for t in range(TPT):
                    nc.scalar.activation(
                        eexp[:, t, :], logits[:, t, :], AF.Exp, bias=negm1[:, t, :]
                    )
                den = gatep.tile([128, TPT, 1], FP32, tag="den")
                nc.vector.tensor_reduce(den, eexp, mybir.AxisListType.X, ALU.add)
                rden = gatep.tile([128, TPT, 1], FP32, tag="rden")
                nc.vector.reciprocal(rden, den)
                probs = gatep.tile([128, TPT, E], FP32, tag="probs")
                nc.vector.tensor_tensor(probs, eexp, rden.to_broadcast([128, TPT, E]), ALU.mult)
                psum_e = gatep.tile([128, E], FP32, tag="psum_e")
                nc.vector.tensor_reduce(
                    psum_e, probs.rearrange("p t e -> p e t"), mybir.AxisListType.X, ALU.add
                )
                nc.vector.tensor_add(imp_acc, imp_acc, psum_e)
                m12 = gatep.tile([128, TPT, E], FP32, tag="m12")
                nc.vector.tensor_add(m12, mask1, mask2)
                cnt_e = gatep.tile([128, E], FP32, tag="cnt_e")
                nc.vector.tensor_reduce(
                    cnt_e, m12.rearrange("p t e -> p e t"), mybir.AxisListType.X, ALU.add
                )
                nc.vector.tensor_add(load_acc, load_acc, cnt_e)

                # ---------- mm1: hT[f, tok] = relu(W1.T x + b1) ----------
                hT = hbuf.tile([128, MF, CHUNK], BF16, tag="hT")
                for m in range(MF):
                    w1t = wstream.tile([128, KD, 128], BF16, tag="w1t")
                    nc.sync.dma_start(w1t, w1bf_d[m])
                    for h in range(2):
                        ps1 = ps_1.tile([128, 512], FP32, tag="ps1")
                        for k in range(KD):
                            nc.tensor.matmul(
                                ps1, w1t[:, k, :], xT[:, k, h * 512:(h + 1) * 512],
                                start=(k == 0), stop=(k == KD - 1),
                            )
                        nc.scalar.activation(
                            hT[:, m, h * 512:(h + 1) * 512], ps1, AF.Relu,
                            bias=b1_part[:, m:m + 1],
                        )

                # ---------- mm2: out[tok, d] = (hT.T W2 + b2) * gate_e ----------
                for tg in range(TPT // TG):
                    for n2 in range(ND2):
                        ps2s = []
                        for _pi in range(TG):
                            ps2_t = ps_2.tile([128, 512], FP32, tag="ps2", name=f"ps2_{_pi}")
                            ps2s.append(ps2_t)
                        for m in range(MF):
                            w2t = wstream.tile([128, 512], BF16, tag="w2t")
                            nc.sync.dma_start(w2t, w2bf_d[n2, m])
                            for tq in range(TG):
                                t = tg * TG + tq
                                nc.tensor.matmul(
                                    ps2s[tq], hT[:, m, t * 128:(t + 1) * 128], w2t,
                                    start=(m == 0), stop=(m == MF - 1),
                                )
                        for tq in range(TG):
                            t = tg * TG + tq
                            ob2 = outp.tile([128, 512], FP32, tag="ob2")
                            nc.vector.tensor_add(ob2, ps2s[tq], b2f_sb[:, n2 * 512:(n2 + 1) * 512])
                            osb = outp.tile([128, 512], FP32, tag="osb")
                            nc.scalar.activation(osb, ob2, AF.Copy, scale=ge[:, t:t + 1])
                            row0 = c * CHUNK + t * 128
                            nc.sync.dma_start(
                                partial_d[row0:row0 + 128, n2 * 512:(n2 + 1) * 512], osb
                            )

            # ---------- aux loss final ----------
            psa = ps_r.tile([1, 2 * E], FP32, tag="psr")
            nc.tensor.matmul(psa[:, :E], ones32_col, imp_acc, start=True, stop=True)
            nc.tensor.matmul(psa[:, E:], ones32_col, load_acc, start=True, stop=True)
            aux_sb = auxp.tile([1, 2 * E], FP32)
            nc.vector.tensor_copy(aux_sb, psa)
            prod = auxp.tile([1, E], FP32)
            nc.vector.tensor_tensor(prod, aux_sb[:, :E], aux_sb[:, E:], ALU.mult)
            auxv = auxp.tile([1, 1], FP32)
            nc.vector.tensor_reduce(auxv, prod, mybir.AxisListType.X, ALU.add)
            nc.vector.tensor_scalar_mul(auxv, auxv, float(E) / (T * T * TOPK))
            nc.sync.dma_start(aux_d[:], auxv)

    # ---------- ReduceScatter + final output, after the Tile barrier ----------
    core_ids = list(range(NCORES))
    with (
        nc.Block() as block,
        nc.semaphore("cc_sem") as cc_sem,
        nc.semaphore("dma_sem") as dma_sem,
    ):
        @block.gpsimd
        def _(gpsimd):
            gpsimd.collective_compute(
                "ReduceScatter",
                ALU.add,
                replica_groups=[core_ids],
                ins=[partial_d[:]],
                outs=[rs_out_d[:]],
            ).then_inc(cc_sem, 1)
            gpsimd.wait_ge(cc_sem, 1)
            gpsimd.dma_start(out=out_d[:], in_=rs_out_d[:]).then_inc(dma_sem, 16)
            gpsimd.wait_ge(dma_sem, 16)

    nc.finalize()
    return nc


_NC_CACHE = None


def _get_nc():
    global _NC_CACHE
    if _NC_CACHE is None:
        _NC_CACHE = build_moe_kernel()
    return _NC_CACHE


def make_in_maps(x, W1, b1, W2, b2, Wr, br):
    x = np.ascontiguousarray(np.asarray(x, np.float32).reshape(T, D))
    in_maps = []
    for r in range(NCORES):
        esel = np.zeros((128, E), np.float32)
        esel[:, r] = 1.0
        in_maps.append({
            "x": x,
            "W1": np.ascontiguousarray(np.asarray(W1[r], np.float32)),
            "b1": np.ascontiguousarray(np.asarray(b1[r], np.float32)),
            "W2": np.ascontiguousarray(np.asarray(W2[r], np.float32)),
            "Wr": np.ascontiguousarray(np.asarray(Wr, np.float32)),
            "brf": np.ascontiguousarray(np.tile(np.asarray(br, np.float32)[None, :], (128, 1))),
            "b2f": np.ascontiguousarray(np.tile(np.asarray(b2[r], np.float32)[None, :], (128, 1))),
            "esel": esel,
        })
    return in_maps


def kernel(x, W1, b1, W2, b2, Wr, br):
    nc = _get_nc()
    in_maps = make_in_maps(x, W1, b1, W2, b2, Wr, br)
    res = run_bass_kernel_spmd(nc, in_maps, list(range(NCORES))).results
    out = np.concatenate([res[r]["out"] for r in range(NCORES)], axis=0)
    aux = np.float32(res[0]["aux"][0, 0])
    return out.reshape(B, S, D), aux
